# revision 31
# baseline (speedup 1.0000x reference)
"""Distributed multi-head attention on 8 Trainium2 NeuronCores (Bass/Tile), v4.

Problem: x[4,2048,1024] f32; q = x@Wq, kv = x@Wkv, 16 heads x 64;
softmax(q k^T / sqrt(64)) @ v; out @ Wo + bo.

Sharding (no collectives): 8 cores = 4 batches x 2 query halves.
K/V computed over the full sequence on both cores of a batch. The host
rolls each core's sequence so its query half is always xt[:, 0:1024]
(attention is permutation-invariant over keys), eliminating the xtq
input.

Layout: per head pair (= Wk/Wq column block m), KT/V projection work is
emitted WITH the pair's attention so the Tile list-scheduler fills the
Act-limited softmax bubbles with projection matmuls.

  QT [inner,qtok]  = Wq^T x_q^T
  KT [inner,tok]   = Wk^T x^T       (per m-block, woven into attention)
  Vg               = x @ Wv         (per pair-quarter, per-head ones
                                     layout [ones|V_even|ones|V_odd])
  scT[j,i-half]    = K_h Q_h^T  per (head, jt, qhalf); even/odd packed
                     into one [128,1024] 2-bank PSUM tile (bufs=2 so the
                     next score matmuls never wait on the exp read)
  at               = exp(scT)   ONE FD=1024 ScalarE call per jt
                     (amortizes the ~172-cycle ACT overhead), f32->bf16
  po[128,half]     = [V_h|1]^T at   accumulated over jt; 64 out + 64 den
  outT             = out * recip(den)
  y                = outT^T Wo, + bo via DVE add on the PSUM->SBUF copy

The jt loop is software-pipelined over PAIRS of jt so the 4 K=64 score
matmuls issue back-to-back, then the 4 K=128 attnV matmuls (fewer PE
array-mode switches).
"""

import os
import sys

for _p in ("/opt/trn_rl_repo", "/root/.axon_site/_ro/trn_rl_repo"):
    if os.path.isdir(_p) and _p not in sys.path:
        sys.path.append(_p)

import numpy as np
import ml_dtypes

import concourse.bacc as bacc
import concourse.mybir as mybir
import concourse.tile as tile
from concourse.bass_utils import run_bass_kernel_spmd
from contextlib import ExitStack

P = 128
DIM = 1024
HEADS = 16
DH = 64
NSEQ = 2048
NTOK = 1024  # query tokens per core (sequence half)
KD = DIM // P  # 8 contraction tiles
MI = DIM // P  # 8 inner tiles
NJT = NSEQ // P  # 16 key tiles
SCALE = 1.0 / DH**0.5
N_CORES = 8

CD = mybir.dt.bfloat16
NP_CD = ml_dtypes.bfloat16
F32 = mybir.dt.float32
Exp = mybir.ActivationFunctionType.Exp
MULT = mybir.AluOpType.mult
ADD = mybir.AluOpType.add
BYPASS = mybir.AluOpType.bypass
RG = [[0, 1], [2, 3], [4, 5], [6, 7]]  # batch pairs exchange K/V halves

_CACHE = {}


def build_nc(unroll=1):
    nc = bacc.Bacc(
        "TRN2", target_bir_lowering=False, debug=False, num_devices=N_CORES
    )

    xt_e = nc.dram_tensor("xt", [DIM, NSEQ], CD, kind="ExternalInput")
    wq_e = nc.dram_tensor("wq", [DIM, DIM], CD, kind="ExternalInput")
    wk_e = nc.dram_tensor("wk", [DIM, DIM], CD, kind="ExternalInput")
    wv_e = nc.dram_tensor("wv", [DIM, DIM], CD, kind="ExternalInput")
    wo_e = nc.dram_tensor("wo", [DIM, DIM], CD, kind="ExternalInput")
    bo_e = nc.dram_tensor("bo", [1, DIM], CD, kind="ExternalInput")
    out_e = nc.dram_tensor("out", [NTOK, DIM], F32, kind="ExternalOutput")

    # DRAM views with the contraction dim on partitions
    xt_r = xt_e.ap().rearrange("(k p) n -> p k n", p=P)
    wq_r = wq_e.ap().rearrange("(k p) n -> p k n", p=P)
    wk_r = wk_e.ap().rearrange("(k p) n -> p k n", p=P)
    wv_r = wv_e.ap().rearrange("(k p) n -> p k n", p=P)
    wo_r = wo_e.ap().rearrange("(k p) n -> p k n", p=P)
    out_r = out_e.ap()

    with tile.TileContext(nc) as tc, ExitStack() as top:
        const = top.enter_context(tc.tile_pool(name="const", bufs=1))
        wo_p = top.enter_context(tc.tile_pool(name="wo_p", bufs=1))
        qt_p = top.enter_context(tc.tile_pool(name="qt_p", bufs=3))
        kt_p = top.enter_context(tc.tile_pool(name="kt_p", bufs=2))
        vg_p = top.enter_context(tc.tile_pool(name="vg_p", bufs=1))
        ot_p = top.enter_context(tc.tile_pool(name="ot_p", bufs=1))
        at_p = top.enter_context(tc.tile_pool(name="at_p", bufs=3))
        rb_p = top.enter_context(tc.tile_pool(name="rb_p", bufs=1))
        y_p = top.enter_context(tc.tile_pool(name="y_p", bufs=2))

        bo_sb = const.tile([1, DIM], CD)
        bo_row = const.tile([1, P], CD)

        wo_sb = wo_p.tile([P, KD, DIM], CD)
        Vg = vg_p.tile([P, NJT, (HEADS // 2) * (4 * DH)], CD)
        outT = ot_p.tile([P, MI, NTOK], CD)

        # per-head ones layout: per pair [ones|V_even|ones|V_odd] so both
        # heads' attnV put the softmax denominator at partition base 0
        # (the custom-DVE reciprocal only runs at base 0 -> no den copy)
        vg4 = Vg.rearrange("p t (pr c) -> p t pr c", c=4 * DH)
        nc.vector.memset(vg4[:, :, :, 0:DH], 1.0)
        nc.vector.memset(vg4[:, :, :, 2 * DH : 3 * DH], 1.0)
        nc.vector.memset(bo_row[:], 1.0)

        def emit_body():
            with ExitStack() as es:
                # PSUM budget (8 banks): sc_p 1 tag x 2 bufs x 2 banks = 4
                # (even+odd head scores side by side -> one FD=1024 exp,
                # double-buffered so score matmuls never wait on exp reads),
                # po_p 2 tags x 1 buf x 1 bank = 2, pb_p 1 tag x 2 bufs = 2
                sc_p = es.enter_context(tc.tile_pool(name="sc_p", bufs=2, space="PSUM"))
                po_p = es.enter_context(tc.tile_pool(name="po_p", bufs=1, space="PSUM"))
                pb_p = es.enter_context(tc.tile_pool(name="pb_p", bufs=2, space="PSUM"))
                b_es = ExitStack()
                b_pool = b_es.enter_context(tc.tile_pool(name="b_pool", bufs=1))
                wm_p = b_es.enter_context(tc.tile_pool(name="wm_p", bufs=2))
                kst_p = b_es.enter_context(tc.tile_pool(name="kst_p", bufs=2))
                vst_p = b_es.enter_context(tc.tile_pool(name="vst_p", bufs=2))
                dram = b_es.enter_context(
                    tc.tile_pool(name="dram", bufs=1, space="DRAM")
                )
                xt_sb = b_pool.tile([P, KD, NSEQ], CD)
                wv_sb = b_pool.tile([P, KD, DIM], CD)


                qts = {}
                kts = {}
                ksts = {}

                def emit_qt(m, xt_dma=None):
                    # stream the m-th column block of Wq; queries are
                    # xt[:, 0:NTOK] (host rolled the sequence)
                    wq_m = wm_p.tile([P, KD, P], CD, name="wq_m", tag="wq_m")
                    for k in range(KD):
                        nc.sync.dma_start(
                            out=wq_m[:, k, :], in_=wq_r[:, k, m * P : (m + 1) * P]
                        )
                        if xt_dma is not None:
                            xt_dma(k)
                    qt = qt_p.tile([P, NTOK], CD, name="qt", tag="qt")
                    qts[m] = qt
                    for n in range(2):
                        ps = pb_p.tile([P, 512], F32, name="ps_a", tag="pb")
                        for k in range(KD):
                            nc.tensor.matmul(
                                ps[:],
                                wq_m[:, k, :],
                                xt_sb[:, k, n * 512 : (n + 1) * 512],
                                start=(k == 0),
                                stop=(k == KD - 1),
                            )
                        nc.vector.tensor_copy(
                            qt[:, n * 512 : (n + 1) * 512], ps[:]
                        )

                def emit_kt(m, xt_dma=None):
                    # stream the m-th column block of Wk. The m=0 prologue
                    # block issues from the Activation HWDGE queue so it
                    # streams in parallel with qt(0)'s SP-queue DMAs.
                    dma_eng = nc.scalar if xt_dma is not None else nc.sync
                    wk_m = wm_p.tile([P, KD, P], CD, name="wk_m", tag="wk_m")
                    for k in range(KD):
                        dma_eng.dma_start(
                            out=wk_m[:, k, :], in_=wk_r[:, k, m * P : (m + 1) * P]
                        )
                        if xt_dma is not None:
                            xt_dma(k)
                    kt = kt_p.tile([P, NSEQ], CD, name="kt", tag="kt")
                    kts[m] = kt
                    for half in range(2):
                        for n in range(2):
                            c0 = half * 1024 + n * 512
                            ps = pb_p.tile([P, 512], F32, name="ps_b", tag="pb")
                            for k in range(KD):
                                nc.tensor.matmul(
                                    ps[:],
                                    wk_m[:, k, :],
                                    xt_sb[:, k, c0 : c0 + 512],
                                    start=(k == 0),
                                    stop=(k == KD - 1),
                                )
                            nc.vector.tensor_copy(kt[:, c0 : c0 + 512], ps[:])

                def emit_kt_loc(m):
                    # project only this core's OWN key half (= query half,
                    # xt cols 0:NTOK) for column block m, into staging
                    wk_m = wm_p.tile([P, KD, P], CD, name="wk_m", tag="wk_m")
                    for k in range(KD):
                        nc.sync.dma_start(
                            out=wk_m[:, k, :], in_=wk_r[:, k, m * P : (m + 1) * P]
                        )
                    kst = kst_p.tile([P, NTOK], CD, name="kst", tag="kst")
                    ksts[m] = kst
                    for n in range(2):
                        ps = pb_p.tile([P, 512], F32, name="ps_b", tag="pb")
                        for k in range(KD):
                            nc.tensor.matmul(
                                ps[:],
                                wk_m[:, k, :],
                                xt_sb[:, k, n * 512 : (n + 1) * 512],
                                start=(k == 0),
                                stop=(k == KD - 1),
                            )
                        nc.vector.tensor_copy(kst[:, n * 512 : (n + 1) * 512], ps[:])

                def emit_kt_coll(ma, mb):
                    # 2-rank AllGather of two staged KT half-blocks; both
                    # cores read back the full sequence in TRUE token order
                    # (rank 0 = true first half, rank 1 = true second half)
                    kd_in = dram.tile(
                        [P, 2 * NTOK], CD, name="kd_in", tag=f"kd_in{ma}"
                    )
                    kd_out = dram.tile(
                        [2 * P, 2 * NTOK], CD, name="kd_out", tag=f"kd_out{ma}"
                    )
                    nc.gpsimd.dma_start(out=kd_in[:, 0:NTOK], in_=ksts[ma][:])
                    nc.gpsimd.dma_start(
                        out=kd_in[:, NTOK : 2 * NTOK], in_=ksts[mb][:]
                    )
                    nc.gpsimd.collective_compute(
                        "AllGather",
                        BYPASS,
                        replica_groups=RG,
                        ins=[kd_in[:].opt()],
                        outs=[kd_out[:].opt()],
                    )
                    kv = kd_out.rearrange("(r p) n -> p r n", p=P)
                    for m, lo in ((ma, 0), (mb, NTOK)):
                        kt = kt_p.tile([P, NSEQ], CD, name="kt", tag="kt")
                        kts[m] = kt
                        for r in range(2):
                            nc.scalar.dma_start(
                                out=kt[:, r * NTOK : (r + 1) * NTOK],
                                in_=kv[:, r, lo : lo + NTOK],
                            )

                def emit_v_loc(q):
                    # V columns for pairs 2q, 2q+1, own token half only,
                    # staged as [two, t, pr, c] so each (rank, two) readback
                    # source is one contiguous run
                    vst = vst_p.tile(
                        [P, 2, NTOK // P, 2, DH], CD, name="vst", tag="vst"
                    )
                    for t in range(NTOK // P):
                        ps = pb_p.tile([P, 512], F32, name="ps_v", tag="pb")
                        for k in range(KD):
                            nc.tensor.matmul(
                                ps[:, 0:256],
                                xt_sb[:, k, t * P : (t + 1) * P],
                                wv_sb[:, k, q * 256 : (q + 1) * 256],
                                start=(k == 0),
                                stop=(k == KD - 1),
                            )
                        ps4 = ps[:, 0:256].rearrange(
                            "p (pr two c) -> p pr two c", pr=2, two=2, c=DH
                        )
                        for tw in range(2):
                            nc.vector.tensor_copy(
                                vst[:, tw, t, :, :], ps4[:, :, tw, :]
                            )
                    vd_in = dram.tile(
                        [P, 2 * NTOK], CD, name="vd_in", tag=f"vd_in{q}"
                    )
                    vd_out = dram.tile(
                        [2 * P, 2 * NTOK], CD, name="vd_out", tag=f"vd_out{q}"
                    )
                    nc.gpsimd.dma_start(
                        out=vd_in[:],
                        in_=vst.rearrange("p two t pr c -> p (two t pr c)"),
                    )
                    nc.gpsimd.collective_compute(
                        "AllGather",
                        BYPASS,
                        replica_groups=RG,
                        ins=[vd_in[:].opt()],
                        outs=[vd_out[:].opt()],
                    )
                    nlt = NTOK // P
                    vv = vd_out.rearrange(
                        "(r p) (two t pr c) -> p r two t pr c",
                        p=P,
                        two=2,
                        t=nlt,
                        pr=2,
                        c=DH,
                    )
                    for r in range(2):
                        for tw in range(2):
                            for pr in range(2):
                                nc.scalar.dma_start(
                                    out=vg4[
                                        :,
                                        r * nlt : (r + 1) * nlt,
                                        2 * q + pr,
                                        (1 + 2 * tw) * DH : (2 + 2 * tw) * DH,
                                    ],
                                    in_=vv[:, r, tw, :, pr, :],
                                )

                def emit_v_quarter(q):
                    # V columns for pairs 2q, 2q+1 (256 wv cols), all seq tiles
                    for t in range(NJT):
                        ps = pb_p.tile([P, 512], F32, name="ps_v", tag="pb")
                        for k in range(KD):
                            nc.tensor.matmul(
                                ps[:, 0:256],
                                xt_sb[:, k, t * P : (t + 1) * P],
                                wv_sb[:, k, q * 256 : (q + 1) * 256],
                                start=(k == 0),
                                stop=(k == KD - 1),
                            )
                        ps_h = ps[:, 0:256].rearrange(
                            "p (pr two c) -> p pr two c", two=2, c=DH
                        )
                        nc.vector.tensor_copy(
                            vg4[:, t, 2 * q : 2 * q + 2, DH : 2 * DH],
                            ps_h[:, :, 0, :],
                        )
                        nc.vector.tensor_copy(
                            vg4[:, t, 2 * q : 2 * q + 2, 3 * DH : 4 * DH],
                            ps_h[:, :, 1, :],
                        )

                def emit_pair(pair):
                    QT = qts[pair]
                    KT = kts[pair]
                    vg_lo = {0: pair * (4 * DH), 1: pair * (4 * DH) + 2 * DH}
                    for qh in range(2):
                        c0 = qh * 512
                        po = {
                            0: po_p.tile([P, 512], F32, name="po_e", tag="po_e"),
                            1: po_p.tile([P, 512], F32, name="po_o", tag="po_o"),
                        }

                        def emit_scores(jt):
                            # both heads' scores into one 2-bank PSUM tile:
                            # even sub at cols 0:512 (bank A), odd at
                            # 512:1024 (bank B); ONE FD=1024 exp amortizes
                            # the ~172-cycle ACT instruction overhead
                            ps = sc_p.tile([P, 1024], F32, name="sc", tag="sc")
                            for sub in range(2):
                                hb = sub * DH
                                nc.tensor.matmul(
                                    ps[:, sub * 512 : (sub + 1) * 512],
                                    KT[hb : hb + DH, jt * P : (jt + 1) * P],
                                    QT[hb : hb + DH, c0 : c0 + 512],
                                    start=True,
                                    stop=True,
                                )
                            at = at_p.tile([P, 1024], CD, name="at", tag="at")
                            nc.scalar.activation(at[:], ps[:], Exp)
                            return at

                        def emit_attnv(jt, at):
                            for sub in range(2):
                                nc.tensor.matmul(
                                    po[sub][:],
                                    Vg[:, jt, vg_lo[sub] : vg_lo[sub] + 2 * DH],
                                    at[:, sub * 512 : (sub + 1) * 512],
                                    start=(jt == 0),
                                    stop=(jt == NJT - 1),
                                )

                        # software pipeline over PAIRS of jt: the 4 K=64
                        # score matmuls (rows 0/64 alternating) issue
                        # back-to-back, then the 4 K=128 attnV matmuls --
                        # fewer PE array-mode switches per jt
                        at_prev = [emit_scores(0), emit_scores(1)]
                        for g in range(NJT // 2):
                            at_next = (
                                [emit_scores(2 * g + 2), emit_scores(2 * g + 3)]
                                if g + 1 < NJT // 2
                                else None
                            )
                            emit_attnv(2 * g, at_prev[0])
                            emit_attnv(2 * g + 1, at_prev[1])
                            at_prev = at_next
                        # normalize: both heads have po rows [den|out];
                        # recip reads den directly at partition base 0
                        for sub in range(2):
                            hb = sub * DH
                            rbc = rb_p.tile(
                                [DH, 512], F32, name=f"rbc{sub}", tag=f"rbc{sub}"
                            )
                            nc.vector.reciprocal_approx_fast(
                                rbc[:], po[sub][0:DH, :]
                            )
                            nc.vector.tensor_tensor(
                                outT[hb : hb + DH, pair, c0 : c0 + 512],
                                po[sub][DH : 2 * DH, :],
                                rbc[:],
                                MULT,
                            )

                # ---- woven A (QT), B (KT, V), C (attention) ----
                # B work is emitted AFTER the pair whose Act-bound bubbles
                # it fills (priority = emission order; C matmuls must win
                # ties or B blocks monopolize the PE and starve Act).
                # Dependencies still order each B block before its consumer.
                # prologue-critical DMA order: per-k interleave of wq block
                # 0 with query-half xt (feeds QT(0) k-by-k), then wk block 0
                # with key-half xt (feeds KT(0)), then wv, wo/bo last
                emit_qt(
                    0,
                    xt_dma=lambda k: nc.sync.dma_start(
                        out=xt_sb[:, k, 0:NTOK], in_=xt_r[:, k, 0:NTOK]
                    ),
                )
                emit_kt(
                    0,
                    xt_dma=lambda k: nc.scalar.dma_start(
                        out=xt_sb[:, k, NTOK:NSEQ], in_=xt_r[:, k, NTOK:NSEQ]
                    ),
                )
                for k in range(KD):
                    nc.sync.dma_start(out=wv_sb[:, k, :], in_=wv_r[:, k, :])
                nc.sync.dma_start(out=bo_sb[:], in_=bo_e.ap())
                for k in range(KD):
                    nc.sync.dma_start(out=wo_sb[:, k, :], in_=wo_r[:, k, :])
                # vq(0) must be EMITTED before pair(0) (program order = data
                # order in Tile), but deprioritized so pair(0)'s score
                # matmuls win the PE and Act never starves.
                with tc.high_priority(offset=-700):
                    emit_v_quarter(0)
                # hybrid K/V: pairs 0-1 + V quarter 0 computed fully
                # locally (local key order, fast start); pairs 2-7 K/V via
                # staged 2-rank AllGathers in true token order, each
                # launched ~2 pairs (~70us) before its consumer so
                # collective latency hides under the attention pipeline
                for pair in range(HEADS // 2):
                    emit_pair(pair)
                    if pair + 1 < MI:
                        emit_qt(pair + 1)
                    if pair == 0:
                        emit_kt(1)
                        emit_kt_loc(2)
                        emit_kt_loc(3)
                        emit_kt_coll(2, 3)
                        emit_v_loc(1)
                    elif pair == 1:
                        emit_kt_loc(4)
                        emit_kt_loc(5)
                        emit_kt_coll(4, 5)
                    elif pair == 2:
                        emit_v_loc(2)
                    elif pair == 3:
                        emit_kt_loc(6)
                        emit_kt_loc(7)
                        emit_kt_coll(6, 7)
                    elif pair == 4:
                        emit_v_loc(3)

                b_es.close()

                # ---- phase D: y = outT^T @ Wo + bo ----
                # bo broadcast tile: ones column x bo row via one matmul
                bo_bc = const.tile([P, DIM], F32, name="bo_bc", tag="bo_bc")
                for n in range(2):
                    bo_bc_ps = pb_p.tile([P, 512], F32, name="bo_ps", tag="pb")
                    nc.tensor.matmul(
                        bo_bc_ps[:],
                        bo_row[:, :],
                        bo_sb[:, n * 512 : (n + 1) * 512],
                        start=True,
                        stop=True,
                    )
                    nc.vector.tensor_copy(
                        bo_bc[:, n * 512 : (n + 1) * 512], bo_bc_ps[:]
                    )

                for t in range(NTOK // P):
                    y = y_p.tile([P, DIM], F32, name="y", tag="y")
                    for n in range(2):
                        cs = slice(n * 512, (n + 1) * 512)
                        ps = pb_p.tile([P, 512], F32, name="ps_d", tag="pb")
                        for k in range(KD):
                            nc.tensor.matmul(
                                ps[:],
                                outT[:, k, t * P : (t + 1) * P],
                                wo_sb[:, k, n * 512 : (n + 1) * 512],
                                start=(k == 0),
                                stop=(k == KD - 1),
                            )
                        nc.vector.tensor_tensor(y[:, cs], ps[:], bo_bc[:, cs], ADD)
                        nc.sync.dma_start(out=out_r[t * P : (t + 1) * P, cs], in_=y[:, cs])

        for _rep in range(unroll):
            emit_body()

    nc.compile()
    return nc


def make_in_maps(x, Wq, Wkv, Wo, bo):
    x = np.asarray(x, dtype=np.float32)
    wq_s = (np.asarray(Wq, dtype=np.float32) * SCALE).astype(NP_CD)
    wk = np.ascontiguousarray(np.asarray(Wkv, np.float32)[:, :DIM]).astype(NP_CD)
    wv = np.ascontiguousarray(np.asarray(Wkv, np.float32)[:, DIM:]).astype(NP_CD)
    wo = np.asarray(Wo, dtype=np.float32).astype(NP_CD)
    bo2 = np.asarray(bo, dtype=np.float32).reshape(1, DIM).astype(NP_CD)

    in_maps = []
    for core in range(N_CORES):
        b, s = core // 2, core % 2
        xt = np.ascontiguousarray(x[b].T).astype(NP_CD)
        if s == 1:
            # roll so this core's query half occupies columns 0:NTOK;
            # attention is permutation-invariant over the key axis
            xt = np.ascontiguousarray(np.roll(xt, -NTOK, axis=1))
        in_maps.append(
            {
                "xt": xt,
                "wq": wq_s,
                "wk": wk,
                "wv": wv,
                "wo": wo,
                "bo": bo2,
            }
        )
    return in_maps


def kernel(x, Wq, Wkv, Wo, bo):
    if "nc" not in _CACHE:
        _CACHE["nc"] = build_nc()
    nc = _CACHE["nc"]
    in_maps = make_in_maps(x, Wq, Wkv, Wo, bo)
    res = run_bass_kernel_spmd(nc, in_maps, core_ids=list(range(N_CORES)))
    out = np.empty((4, NSEQ, DIM), dtype=np.float32)
    for core in range(N_CORES):
        b, s = core // 2, core % 2
        out[b, s * NTOK : (s + 1) * NTOK, :] = res.results[core]["out"]
    return out


if __name__ == "__main__":
    rng = np.random.default_rng(0)
    x = rng.standard_normal((4, NSEQ, DIM), dtype=np.float32)
    Wq = rng.standard_normal((DIM, DIM), dtype=np.float32) / 32
    Wkv = rng.standard_normal((DIM, 2 * DIM), dtype=np.float32) / 32
    Wo = rng.standard_normal((DIM, DIM), dtype=np.float32) / 32
    bo = rng.standard_normal((DIM,), dtype=np.float32) * 0.01
    out = kernel(x=x, Wq=Wq, Wkv=Wkv, Wo=Wo, bo=bo)
    print("out", out.shape, out.dtype, np.abs(out).mean())
